# revision 19
# baseline (speedup 1.0000x reference)
"""Trainium2 Bass kernel for nn_BCNLayer (locally-connected 7x7 lattice layer + sigmoid).

Math: y[i,j,b] = sigmoid( sum_{dy,dx in [-3,3]} w[dy+3,dx+3][(i-dy)*W + (j-dx)]
                          * x[(i-dy)*W + (j-dx), b] )   (zero outside lattice)

Strategy (v2, packed-output matmuls):
  - 8-way shard over lattice rows (16 dest rows/core, 22 source rows with halo).
  - PSUM block = [128 partitions = (4 dest rows x 32 dest cols), 512 batch].
    Contraction packs (3 source rows x 38 source cols) = 114 <= 128, so one
    block needs only 4 accumulating matmuls (vs 7 in the row-at-a-time
    scheme): 128 matmuls x 512 cycles = 65.5K PE cycles/core.
  - x is zero-padded on host to [22, 134, B] f16; per jd-block-of-32 a
    [114, 8*1024] window is DMA'd (plain affine pattern).
  - Band weights are packed on host into wl[22, 38, 4, 7, 32] f16 holding only
    the valid dest-row slices; DMA'd into a zeroed lhsT tile with contiguous
    448B runs (~1.5 MB/core instead of 3.7 MB dense).
  - Sigmoid on ACT writes bf16 (halves output traffic; bf16's 8-bit exponent
    keeps tiny sigmoid tails representable, unlike f16).
"""

import os

import numpy as np

H = 128
W = 128
HW = H * W
B = 1024
NCORES = 8
T = H // NCORES      # dest rows per core = 16
SR = T + 6           # source rows per core (halo 3 each side) = 22
NG = 8               # row groups of 3 source rows (last group has 1 row)
NJ = 4               # jd blocks of 32
JB = 32              # jd block width
WIN = JB + 6         # js window per jd block = 38
KP = 3 * WIN         # valid contraction rows = 114 (padded to 128 for FWL)
BC = 512             # batch chunk (psum free dim)
NCH = B // BC        # = 2
NT0 = 4              # dest-row blocks of 4
GSLOT = 3 * 7 + 1    # (g,rho) slots: g<7 x rho<3, plus (g=7,rho=0) = 22

# Compact lhsT: per row-group only the dest-row span actually touched by the
# t0 blocks that use it. TSPAN[g] = (tbase, nslots); offsets are prefix sums.
TSPAN = [(0, 4), (0, 8), (0, 12), (0, 12), (4, 12), (8, 8), (12, 4), (12, 4)]
GOFF = [0]
for _tb, _ns in TSPAN:
    GOFF.append(GOFF[-1] + _ns)
LW = GOFF[-1] * JB   # lhsT free elems per j0 block = 64*32 = 2048

_cache: dict = {}

# filled by the last kernel() call when KERNEL_TRACE=1
last_exec_time_ns = None
last_results = None


def _gslots():
    """(slot, g, rho, r_local, tlo, thi) for each (row-group, in-group-row)."""
    out = []
    for i in range(GSLOT):
        g, rho = (7, 0) if i == 21 else divmod(i, 3)
        r = i - 3  # local source row = 3*g + rho - 3
        tlo = max(0, r - 3)
        thi = min(T, r + 4)
        out.append((i, g, rho, r, tlo, thi))
    return out


def _groups_for_t0(t0):
    """Row groups overlapping dest rows [t0, t0+4)."""
    gs = []
    for g in range(NG):
        glo = 3 * g - 3
        ghi = glo + (1 if g == 7 else 3) - 1
        if ghi >= t0 - 3 and glo <= t0 + 3 + 3:
            gs.append(g)
    return gs


def _build_program():
    from contextlib import ExitStack

    import concourse.bacc as bacc
    import concourse.mybir as mybir
    import concourse.tile as tile

    nc = bacc.Bacc(
        "TRN2", target_bir_lowering=False, debug=False, num_devices=NCORES
    )
    f16 = mybir.dt.float16
    # Pre-windowed x: [j0, (rho*38+js'), g, b] so each window load is one
    # full-partition contiguous DMA (38-partition DMAs only engage ~5 of 16
    # SDMA engines and ran at ~45 GB/s).
    xs = nc.dram_tensor("xs", [NJ, 128, NG, B], f16, kind="ExternalInput").ap()
    # Compact lhsT image per j0 block: [j0, (rho*38+js'), goff(g)+tau*32+dj'].
    wl = nc.dram_tensor("wl", [NJ, 128, LW], f16, kind="ExternalInput").ap()
    # Block layout [t0, j0, (dt*32+dj), b]; host un-permutes to [T, W, B].
    y = nc.dram_tensor(
        "y", [NT0, NJ, 128, B], mybir.dt.bfloat16, kind="ExternalOutput"
    ).ap()

    with tile.TileContext(nc) as tc, ExitStack() as ctx:
        wpool = ctx.enter_context(tc.tile_pool(name="w", bufs=1))
        xpool = ctx.enter_context(tc.tile_pool(name="x", bufs=4))
        ppool = ctx.enter_context(tc.tile_pool(name="ps", bufs=6, space="PSUM"))
        opool = ctx.enter_context(tc.tile_pool(name="o", bufs=6))

        # lhsT layout: free = j0*LW + GOFF[g]*32 + (t - tbase(g))*32 + dj'
        lhsT = wpool.tile([128, NJ * LW], f16, tag="lhsT",
                  padded_shape=[128, NJ * LW + 256])
        lt4 = lhsT[:].rearrange("p (j r) -> p j r", j=NJ)

        # Warm the sigmoid ACT table early. Lives in the persistent pool so
        # its SBUF slot is never recycled (recycling tripped a race with xw).
        warm = wpool.tile([128, 1], mybir.dt.float32, tag="warm",
                  padded_shape=[128, 128])
        nc.vector.memset(warm[:], 0.0)
        nc.scalar.activation(warm[:], warm[:], mybir.ActivationFunctionType.Sigmoid)

        # Descriptor shaping: split the innermost run into 1KB descriptors
        # (the pattern the DGE demonstrably spreads over all engines at
        # ~20 GB/s each); one dma_start per j0 for lhsT (no ring stalls).
        def load_lhsT(j0i, eng):
            eng.dma_start(
                out=lt4[:, j0i].rearrange("p (c k) -> p c k", k=512),
                in_=wl[j0i].rearrange("p (c k) -> p c k", k=512),
            )

        # All 4 windows stay resident (bufs=4) so every window DMA queues
        # immediately at full depth. Window 0's early groups fan out over
        # three queues to cut the head; lhsT j0=0 leads the sync queue.
        def load_xw(xw, j0i, g, eng):
            eng.dma_start(
                out=xw[:, g * B : (g + 1) * B].rearrange(
                    "p (c k) -> p c k", k=512
                ),
                in_=xs[j0i, :, g].rearrange("p (c k) -> p c k", k=512),
            )

        load_lhsT(0, nc.sync)
        xwins = [
            xpool.tile([128, NG * B], f16, tag="xw", name=f"xw{j}",
                       padded_shape=[128, NG * B + 256])
            for j in range(NJ)
        ]
        w0_eng = [nc.gpsimd, nc.scalar, nc.gpsimd, nc.scalar,
                  nc.sync, nc.sync, nc.sync, nc.sync]
        for g in range(NG):
            load_xw(xwins[0], 0, g, w0_eng[g])
        for j0i in range(1, NJ):
            load_lhsT(j0i, nc.gpsimd)
        for j0i in range(1, NJ):
            for g in range(NG):
                load_xw(xwins[j0i], j0i, g, nc.gpsimd if g < 4 else nc.scalar)

        # Main loop: 2 batch chunks x 4 t0 blocks per jd block.
        for j0i in range(NJ):
            xw = xwins[j0i]
            for ch in range(NCH):
                for t0i in range(NT0):
                    t0 = t0i * 4
                    gs = _groups_for_t0(t0)
                    ps = ppool.tile([128, BC], mybir.dt.float32, tag="ps")
                    for gi, g in enumerate(gs):
                        f0 = (GOFF[g] + t0 - TSPAN[g][0]) * JB
                        lhs = lt4[:, j0i, f0 : f0 + 128]
                        rhs = xw[:, g * B + ch * BC : g * B + (ch + 1) * BC]
                        nc.tensor.matmul(
                            ps[:], lhs, rhs,
                            start=(gi == 0), stop=(gi == len(gs) - 1),
                        )
                    ob = opool.tile([128, BC], mybir.dt.bfloat16, tag="ob",
                    padded_shape=[128, BC + 256])
                    nc.scalar.activation(
                        ob[:], ps[:], mybir.ActivationFunctionType.Sigmoid
                    )
                    oeng = nc.sync if j0i < 2 else nc.scalar
                    oeng.dma_start(
                        out=y[t0i, j0i, :, ch * BC : (ch + 1) * BC],
                        in_=ob[:],
                    )
    nc.compile()
    return nc


def _pack_inputs(x: np.ndarray, weights: np.ndarray):
    """Per-core padded x slabs and packed weight slices."""
    x3 = x.reshape(H, W, B)
    w4 = weights.reshape(7, 7, H, W)

    jsp = np.arange(WIN)
    djp = np.arange(JB)
    dxi = djp[None, :] - jsp[:, None] + 6          # [38, 32]
    vdx = (dxi >= 0) & (dxi < 7)
    dxi_c = np.clip(dxi, 0, 6)

    in_maps = []
    for q in range(NCORES):
        xp = np.zeros((SR, W + 6, B), np.float16)
        r0 = T * q - 3
        lo = max(0, -r0)
        hi = min(SR, H - r0)
        xp[lo:hi, 3 : W + 3] = x3[r0 + lo : r0 + hi].astype(np.float16)

        # Pre-windowed x: xsq[j0, rho*38+js', g, b] = xp[3g+rho, j0*32+js', b]
        xsq = np.zeros((NJ, 128, NG, B), np.float16)
        for rho in range(3):
            ng = NG if rho == 0 else NG - 1
            rows = xp[rho : rho + 3 * (ng - 1) + 1 : 3]   # [ng, 134, B]
            for j0i in range(NJ):
                xsq[j0i, rho * WIN : (rho + 1) * WIN, :ng] = rows[
                    :, j0i * JB : j0i * JB + WIN
                ].transpose(1, 0, 2)

        # Compact lhsT image: wlq[j0, rho*38+js', GOFF[g]*32+(t-tbase)*32+dj']
        wlq = np.zeros((NJ, 128, LW), np.float16)
        for (i, g, rho, r, tlo, thi) in _gslots():
            rg = T * q + r
            if not (0 <= rg < H) or thi <= tlo:
                continue
            tb, ns = TSPAN[g]
            for j0i in range(NJ):
                js_g = JB * j0i + jsp - 3          # [38]
                vjs = (js_g >= 0) & (js_g < W)
                js_c = np.clip(js_g, 0, W - 1)
                for tt in range(thi - tlo):
                    t = tlo + tt
                    if not (tb <= t < tb + ns):
                        continue
                    dy = t - r
                    row = w4[dy + 3, :, rg, :]     # [7, W]
                    vals = row[dxi_c, js_c[:, None]]
                    f0 = (GOFF[g] + (t - tb)) * JB
                    wlq[
                        j0i, rho * WIN : (rho + 1) * WIN, f0 : f0 + JB
                    ] = np.where(vdx & vjs[:, None], vals, 0.0).astype(
                        np.float16
                    )
        in_maps.append({"xs": xsq, "wl": wlq})
    return in_maps


def kernel(x: np.ndarray, weights: np.ndarray) -> np.ndarray:
    global last_exec_time_ns, last_results
    from concourse.bass_utils import run_bass_kernel_spmd

    x = np.ascontiguousarray(x, dtype=np.float32)
    weights = np.ascontiguousarray(weights, dtype=np.float32)

    if "v2" not in _cache:
        _cache["v2"] = _build_program()
    nc = _cache["v2"]

    in_maps = _pack_inputs(x, weights)

    trace = os.environ.get("KERNEL_TRACE", "0") == "1"
    res = run_bass_kernel_spmd(
        nc, in_maps, core_ids=list(range(NCORES)), trace=trace
    )
    last_exec_time_ns = res.exec_time_ns
    last_results = res
    outs = []
    for r in res.results:
        yb = np.asarray(r["y"])  # [NT0, NJ, 128, B] block layout
        yb = yb.reshape(NT0, NJ, 4, JB, B).transpose(0, 2, 1, 3, 4)
        outs.append(yb.reshape(T * W, B))
    return np.concatenate(outs, axis=0).astype(np.float32)
